# revision 6
# baseline (speedup 1.0000x reference)
"""Trainium2 Bass kernel for nn_Attention_2851858284976.

Dense transformer attention block, b=8 n=1024 dim=1024 heads=16.
Sharding: pure data parallel — one batch element per NeuronCore (8 cores).

Per-core math (batch element x of shape (n, dim)):
  Y = x @ w_qkv^T                              (n, 3*dim)
  Z = Y.reshape(49152, 64)   # raw reshape: rows are (token, col-block) pairs
  Q = Z[0:16384], K = Z[16384:32768], V = Z[32768:49152], each (16, 1024, 64)
  per head: P^T = exp(scale * K_h @ Q_h^T)     (softmax over the partition axis)
            [O^T; Zs] = [V_h | 1]^T @ P^T      (ones column gives softmax denom)
            oT_h = O^T * (1/Zs broadcast)
  out = (oT stacked).T @ w_out^T + b_out

All matmuls run as float32r (tf32-rate on the PE: 1 cyc/row at N>=256).
Host pre-transposes x / w_qkv / w_out so the contraction dim lands on the
SBUF partition axis; the only on-device transposes are the per-head Q/K
64x1024 blocks (PE transpose via identity).
"""
import numpy as np

import concourse.bass as bass
import concourse.mybir as mybir
from concourse import bacc
from concourse.tile import TileContext
from concourse.bass_utils import run_bass_kernel_spmd
from concourse.masks import make_identity

N_CORES = 8
N = 1024          # tokens
DIM = 1024
E3 = 3 * DIM      # qkv projection width
H = 16            # heads
HD = 64           # head dim
SCALE = HD ** -0.5

F32 = mybir.dt.float32
F32R = mybir.dt.float32r
FT = mybir.ActivationFunctionType


def build():
    nc = bacc.Bacc("TRN2", target_bir_lowering=False, num_devices=N_CORES)
    xt = nc.declare_dram_parameter("xt", [DIM, N], F32R, isOutput=False)
    wqkvt = nc.declare_dram_parameter("wqkvt", [DIM, E3], F32R, isOutput=False)
    woutt = nc.declare_dram_parameter("woutt", [DIM, DIM], F32R, isOutput=False)
    bias = nc.declare_dram_parameter("bias", [1, DIM], F32, isOutput=False)
    outp = nc.declare_dram_parameter("out", [N, DIM], F32, isOutput=True)

    with TileContext(nc) as tc:
        with tc.tile_pool(name="dram", bufs=1, space="DRAM") as dpool, \
             tc.tile_pool(name="singles", bufs=1) as singles:
            ybuf = dpool.tile([N, E3], F32R)
            zv = ybuf.rearrange("a (b c) -> (a b) c", c=HD)   # (49152, 64)

            ident_f = singles.tile([128, 128], F32)
            make_identity(nc, ident_f)
            ident = singles.tile([128, 128], F32R)
            nc.vector.tensor_copy(ident, ident_f)
            oT = singles.tile([128, 8, N], F32R)    # [64*(h%2)+dd, h//2, i]
            biasrep = singles.tile([128, DIM], F32)
            nc.sync.dma_start(out=biasrep, in_=bias[:].to_broadcast((128, DIM)))

            # [V | ones*64] stationary tiles for the PV matmul (ones half gives
            # the softmax denominator replicated on out rows 64-127). The ones
            # half never changes; fill it once in two manually double-buffered
            # tiles (f32r memset is not a legal ISA op, so go via f32 copy).
            ones_f = singles.tile([128, 8, HD], F32)
            nc.vector.memset(ones_f, 1.0)
            vh0 = singles.tile([128, 8, 2 * HD], F32R)
            vh1 = singles.tile([128, 8, 2 * HD], F32R)
            vhs = [vh0, vh1]
            for v in vhs:
                nc.vector.tensor_copy(v[:, :, HD:2 * HD], ones_f)

            # ---------- phase 1: Y = x @ w_qkv^T -> ybuf ----------
            with tc.tile_pool(name="p1", bufs=1) as p1, \
                 tc.tile_pool(name="p1st", bufs=4) as p1st, \
                 tc.tile_pool(name="ps1", bufs=4, space="PSUM") as ps1:
                XT = p1.tile([128, 8, N], F32R)
                nc.sync.dma_start(out=XT, in_=xt[:].rearrange("(a p) i -> p a i", p=128))
                WT = p1.tile([128, 8, E3], F32R)
                nc.sync.dma_start(out=WT, in_=wqkvt[:].rearrange("(a p) e -> p a e", p=128))
                for it in range(8):
                    for ec in range(6):
                        ps = ps1.tile([128, 512], F32)
                        for kt in range(8):
                            nc.tensor.matmul(
                                ps,
                                lhsT=XT[:, kt, it * 128:(it + 1) * 128],
                                rhs=WT[:, kt, ec * 512:(ec + 1) * 512],
                                start=(kt == 0), stop=(kt == 7))
                        st = p1st.tile([128, 512], F32R)
                        nc.vector.tensor_copy(st, ps)
                        nc.sync.dma_start(
                            out=ybuf[it * 128:(it + 1) * 128, ec * 512:(ec + 1) * 512],
                            in_=st)

            # ---------- phases 2+3 SBUF for out-projection ----------
            with tc.tile_pool(name="p3", bufs=1) as p3:
                WOT = p3.tile([128, 8, DIM], F32R)
                nc.sync.dma_start(out=WOT, in_=woutt[:].rearrange("(a p) e -> p a e", p=128))

                # ---------- phase 2: attention per head ----------
                with tc.tile_pool(name="qk", bufs=2) as qkpool, \
                     tc.tile_pool(name="raw", bufs=3) as rawpool, \
                     tc.tile_pool(name="pt", bufs=6) as ptpool, \
                     tc.tile_pool(name="rz", bufs=3) as rzpool, \
                     tc.tile_pool(name="tps", bufs=2, space="PSUM") as tpsum, \
                     tc.tile_pool(name="sps", bufs=3, space="PSUM") as spsum, \
                     tc.tile_pool(name="ops", bufs=2, space="PSUM") as opsum:
                    for h in range(H):
                        qt = qkpool.tile([64, N], F32R, tag="qt")
                        ktr = qkpool.tile([64, N], F32R, tag="ktr")
                        for dst, base in ((qt, h * N), (ktr, 16384 + h * N)):
                            raw = rawpool.tile([128, 8, HD], F32R, tag="raw")
                            nc.sync.dma_start(
                                out=raw,
                                in_=zv[base:base + N].rearrange("(t p) d -> p t d", p=128))
                            for t in range(8):
                                tp = tpsum.tile([64, 128], F32R)
                                nc.tensor.transpose(
                                    tp,
                                    raw[:, t, :],
                                    ident)
                                nc.vector.tensor_copy(dst[:, t * 128:(t + 1) * 128], tp)
                        vh = vhs[h % 2]
                        nc.sync.dma_start(
                            out=vh[:, :, 0:HD],
                            in_=zv[32768 + h * N: 32768 + (h + 1) * N].rearrange(
                                "(t p) d -> p t d", p=128))

                        po, fo = 64 * (h % 2), h // 2
                        for ic in range(2):
                            ops = opsum.tile([128, 512], F32)
                            for jt in range(8):
                                sps = spsum.tile([128, 512], F32)
                                nc.tensor.matmul(
                                    sps,
                                    lhsT=ktr[:, jt * 128:(jt + 1) * 128],
                                    rhs=qt[:, ic * 512:(ic + 1) * 512],
                                    start=True, stop=True)
                                pt = ptpool.tile([128, 512], F32R, tag="pt")
                                nc.scalar.activation(pt, sps, FT.Exp, scale=SCALE)
                                nc.tensor.matmul(
                                    ops,
                                    lhsT=vh[:, jt, :],
                                    rhs=pt,
                                    start=(jt == 0), stop=(jt == 7),
                                    skip_group_check=True)
                            rzs = rzpool.tile([64, 512], F32, tag="rzs")
                            nc.vector.reciprocal(rzs, ops[64:128, :])
                            nc.vector.tensor_mul(
                                oT[po:po + 64, fo, ic * 512:(ic + 1) * 512],
                                ops[0:64, :], rzs)

                # ---------- phase 3: out = oT.T @ w_out^T + b ----------
                with tc.tile_pool(name="p3st", bufs=4) as p3st, \
                     tc.tile_pool(name="ps3", bufs=4, space="PSUM") as ps3:
                    for it in range(8):
                        for ec in range(2):
                            rps = ps3.tile([128, 512], F32)
                            for ct in range(8):
                                nc.tensor.matmul(
                                    rps,
                                    lhsT=oT[:, ct, it * 128:(it + 1) * 128],
                                    rhs=WOT[:, ct, ec * 512:(ec + 1) * 512],
                                    start=(ct == 0), stop=(ct == 7))
                            ost = p3st.tile([128, 512], F32)
                            nc.vector.tensor_add(
                                ost, rps, biasrep[:, ec * 512:(ec + 1) * 512])
                            nc.sync.dma_start(
                                out=outp[it * 128:(it + 1) * 128, ec * 512:(ec + 1) * 512],
                                in_=ost)

    nc.finalize()
    return nc


_CACHE = {}


def _get_nc():
    if "nc" not in _CACHE:
        _CACHE["nc"] = build()
    return _CACHE["nc"]


def make_in_maps(x, w_qkv, w_out, b_out):
    wqkvt = np.ascontiguousarray(np.asarray(w_qkv, dtype=np.float32).T)
    woutt = np.ascontiguousarray(np.asarray(w_out, dtype=np.float32).T)
    bias = np.ascontiguousarray(np.asarray(b_out, dtype=np.float32).reshape(1, DIM))
    x = np.asarray(x, dtype=np.float32)
    return [
        {
            "xt": np.ascontiguousarray(x[b].T),
            "wqkvt": wqkvt,
            "woutt": woutt,
            "bias": bias,
        }
        for b in range(N_CORES)
    ]


def kernel(x, w_qkv, w_out, b_out):
    nc = _get_nc()
    in_maps = make_in_maps(x, w_qkv, w_out, b_out)
    res = run_bass_kernel_spmd(nc, in_maps, core_ids=list(range(N_CORES)))
    return np.stack(
        [res.results[b]["out"] for b in range(N_CORES)], axis=0
    ).astype(np.float32)


# revision 9
# speedup vs baseline: 1.6954x; 1.6954x over previous
"""Trainium2 Bass kernel for nn_Attention_2851858284976.

Dense transformer attention block, b=8 n=1024 dim=1024 heads=16.
Sharding: pure data parallel — one batch element per NeuronCore (8 cores).

Per-core math (batch element x of shape (n, dim)):
  Y = x @ w_qkv^T                              (n, 3*dim)
  Z = Y.reshape(49152, 64)   # raw reshape: rows are (token, col-block) pairs
  Q = Z[0:16384], K = Z[16384:32768], V = Z[32768:49152], each (16, 1024, 64)
  per head: P^T = exp(scale * K_h @ Q_h^T)     (softmax along the partition axis)
            [O^T; Zs*64] = [V_h | 1*64]^T @ P^T  (ones cols replicate the denom)
            oT_h = O^T * (1/Zs)
  out = (oT stacked).T @ w_out^T + b_out

Matmul datapath is bf16 (1 cyc/row on the PE); accumulation fp32 in PSUM.
Host pre-transposes x / w_qkv / w_out so the contraction dim lands on the
SBUF partition axis. Q/K head blocks are PE-transposed in one dense block
right after the projection (keeps the PE HAM-warm through attention), with
even heads on partitions 0-63 and odd heads on 64-127 so score matmuls of a
head pair run concurrently in different PE row groups.
"""
import numpy as np
import ml_dtypes

import concourse.bass as bass
import concourse.mybir as mybir
from concourse import bacc
from concourse.tile import TileContext
from concourse.bass_utils import run_bass_kernel_spmd
from concourse.masks import make_identity

N_CORES = 8
N = 1024          # tokens
DIM = 1024
E3 = 3 * DIM      # qkv projection width
H = 16            # heads
HD = 64           # head dim
SCALE = HD ** -0.5

F32 = mybir.dt.float32
BF = mybir.dt.bfloat16
FT = mybir.ActivationFunctionType


def build():
    nc = bacc.Bacc("TRN2", target_bir_lowering=False, num_devices=N_CORES)
    xt = nc.declare_dram_parameter("xt", [DIM, N], BF, isOutput=False)
    wqkvt = nc.declare_dram_parameter("wqkvt", [DIM, E3], BF, isOutput=False)
    woutt = nc.declare_dram_parameter("woutt", [DIM, DIM], BF, isOutput=False)
    bias = nc.declare_dram_parameter("bias", [1, DIM], F32, isOutput=False)
    outp = nc.declare_dram_parameter("out", [N, DIM], F32, isOutput=True)

    with TileContext(nc) as tc:
        with tc.tile_pool(name="dram", bufs=1, space="DRAM") as dpool, \
             tc.tile_pool(name="singles", bufs=1) as singles:
            ybuf = dpool.tile([N, E3], BF)
            zv = ybuf.rearrange("a (b c) -> (a b) c", c=HD)   # (49152, 64)

            ident_f = singles.tile([128, 128], F32)
            make_identity(nc, ident_f)
            ident = singles.tile([128, 128], BF)
            nc.vector.tensor_copy(ident, ident_f)
            oT = singles.tile([128, 8, N], BF)    # [64*(h%2)+dd, h//2, i]
            biasrep = singles.tile([128, DIM], F32)
            nc.sync.dma_start(out=biasrep, in_=bias[:].to_broadcast((128, DIM)))

            # [V | ones*64] stationary tiles for the PV matmul; ones half gives
            # the softmax denominator replicated on out rows 64-127. The ones
            # half never changes; fill once in 4 manually rotated tiles.
            ones_f = singles.tile([128, 8, HD], F32)
            nc.vector.memset(ones_f, 1.0)
            vh0 = singles.tile([128, 8, 2 * HD], BF)
            vh1 = singles.tile([128, 8, 2 * HD], BF)
            vh2 = singles.tile([128, 8, 2 * HD], BF)
            vh3 = singles.tile([128, 8, 2 * HD], BF)
            vhs = [vh0, vh1, vh2, vh3]
            for v in vhs:
                nc.vector.tensor_copy(v[:, :, HD:2 * HD], ones_f)

            # ---------- phase 1: Y = x @ w_qkv^T -> ybuf ----------
            with tc.tile_pool(name="p1", bufs=1) as p1, \
                 tc.tile_pool(name="p1st", bufs=4) as p1st, \
                 tc.tile_pool(name="ps1", bufs=4, space="PSUM") as ps1:
                XT = p1.tile([128, 8, N], BF)
                nc.sync.dma_start(out=XT, in_=xt[:].rearrange("(a p) i -> p a i", p=128))
                WT = p1.tile([128, 8, E3], BF)
                nc.sync.dma_start(out=WT, in_=wqkvt[:].rearrange("(a p) e -> p a e", p=128))
                for it in range(8):
                    for ec in range(6):
                        ps = ps1.tile([128, 512], F32)
                        for kt in range(8):
                            nc.tensor.matmul(
                                ps,
                                lhsT=XT[:, kt, it * 128:(it + 1) * 128],
                                rhs=WT[:, kt, ec * 512:(ec + 1) * 512],
                                start=(kt == 0), stop=(kt == 7))
                        st = p1st.tile([128, 512], BF)
                        nc.scalar.copy(st, ps)
                        nc.sync.dma_start(
                            out=ybuf[it * 128:(it + 1) * 128, ec * 512:(ec + 1) * 512],
                            in_=st)

            with tc.tile_pool(name="p3", bufs=1) as p3:
                WOT = p3.tile([128, 8, DIM], BF)
                nc.sync.dma_start(out=WOT, in_=woutt[:].rearrange("(a p) e -> p a e", p=128))

                with tc.tile_pool(name="qk", bufs=1) as qkpool, \
                     tc.tile_pool(name="raw", bufs=4) as rawpool:
                    QT = qkpool.tile([128, 8, N], BF)
                    KT = qkpool.tile([128, 8, N], BF)

                    # dense PE-transpose block: all heads' Q/K -> (64,1024) each
                    with tc.tile_pool(name="tps", bufs=4, space="PSUM") as tpsum:
                        for h in range(H):
                            po, hf = 64 * (h % 2), h // 2
                            for dst, base in ((QT, h * N), (KT, 16384 + h * N)):
                                raw = rawpool.tile([128, 8, HD], BF, tag="raw")
                                nc.sync.dma_start(
                                    out=raw,
                                    in_=zv[base:base + N].rearrange(
                                        "(t p) d -> p t d", p=128))
                                for g in range(2):
                                    tpp = tpsum.tile([64, 512], BF)
                                    for t in range(4):
                                        nc.tensor.transpose(
                                            tpp[:, t * 128:(t + 1) * 128],
                                            raw[:, g * 4 + t, :],
                                            ident)
                                    eng = nc.vector if (h + g) % 2 == 0 else nc.scalar
                                    if eng is nc.vector:
                                        nc.vector.tensor_copy(
                                            dst[po:po + 64, hf, g * 512:(g + 1) * 512],
                                            tpp)
                                    else:
                                        nc.scalar.copy(
                                            dst[po:po + 64, hf, g * 512:(g + 1) * 512],
                                            tpp)

                    # ---------- phase 2: attention per head ----------
                    # Even heads sit on partitions 0-63, odd on 64-127, so
                    # score matmuls of adjacent heads land in different PE row
                    # groups and can overlap at head boundaries.
                    with tc.tile_pool(name="pt", bufs=4) as ptpool, \
                         tc.tile_pool(name="rz", bufs=4) as rzpool, \
                         tc.tile_pool(name="sps", bufs=2, space="PSUM") as spsum, \
                         tc.tile_pool(name="ops", bufs=2, space="PSUM") as opsum:
                        for h in range(H):
                            po, hf = 64 * (h % 2), h // 2
                            vh = vhs[h % 4]
                            nc.sync.dma_start(
                                out=vh[:, :, 0:HD],
                                in_=zv[32768 + h * N: 32768 + (h + 1) * N
                                       ].rearrange("(t p) d -> p t d", p=128))
                            ops0 = opsum.tile([128, 512], F32, tag="ops0")
                            ops1 = opsum.tile([128, 512], F32, tag="ops1")
                            ops = (ops0, ops1)
                            for jt in range(8):
                                sps = spsum.tile([128, 2, 512], F32, tag="sps")
                                for ic in range(2):
                                    nc.tensor.matmul(
                                        sps[:, ic, :],
                                        lhsT=KT[po:po + 64, hf, jt * 128:(jt + 1) * 128],
                                        rhs=QT[po:po + 64, hf, ic * 512:(ic + 1) * 512],
                                        start=True, stop=True)
                                pt = ptpool.tile([128, 2, 512], BF, tag="pt")
                                nc.scalar.activation(pt, sps, FT.Exp, scale=SCALE)
                                for ic in range(2):
                                    nc.tensor.matmul(
                                        ops[ic],
                                        lhsT=vh[:, jt, :],
                                        rhs=pt[:, ic, :],
                                        start=(jt == 0), stop=(jt == 7),
                                        skip_group_check=True)
                            for ic in range(2):
                                # custom-DVE reciprocal can't read PSUM; stage
                                # the denominator rows through SBUF first.
                                zst = rzpool.tile([64, 512], F32, tag="zst")
                                nc.vector.tensor_copy(zst, ops[ic][64:128, :])
                                rzs = rzpool.tile([64, 512], F32, tag="rzs")
                                nc.vector.reciprocal_approx_fast(rzs, zst)
                                nc.vector.tensor_mul(
                                    oT[po:po + 64, hf, ic * 512:(ic + 1) * 512],
                                    ops[ic][0:64, :], rzs)

                # ---------- phase 3: out = oT.T @ w_out^T + b ----------
                with tc.tile_pool(name="p3st", bufs=4) as p3st, \
                     tc.tile_pool(name="ps3", bufs=4, space="PSUM") as ps3:
                    for it in range(8):
                        for ec in range(2):
                            rps = ps3.tile([128, 512], F32)
                            for ct in range(8):
                                nc.tensor.matmul(
                                    rps,
                                    lhsT=oT[:, ct, it * 128:(it + 1) * 128],
                                    rhs=WOT[:, ct, ec * 512:(ec + 1) * 512],
                                    start=(ct == 0), stop=(ct == 7))
                            ost = p3st.tile([128, 512], F32)
                            nc.vector.tensor_add(
                                ost, rps, biasrep[:, ec * 512:(ec + 1) * 512])
                            nc.sync.dma_start(
                                out=outp[it * 128:(it + 1) * 128,
                                         ec * 512:(ec + 1) * 512],
                                in_=ost)

    nc.finalize()
    return nc


_CACHE = {}


def _get_nc():
    if "nc" not in _CACHE:
        _CACHE["nc"] = build()
    return _CACHE["nc"]


def make_in_maps(x, w_qkv, w_out, b_out):
    bf = ml_dtypes.bfloat16
    wqkvt = np.ascontiguousarray(np.asarray(w_qkv, dtype=np.float32).T).astype(bf)
    woutt = np.ascontiguousarray(np.asarray(w_out, dtype=np.float32).T).astype(bf)
    bias = np.ascontiguousarray(np.asarray(b_out, dtype=np.float32).reshape(1, DIM))
    x = np.asarray(x, dtype=np.float32)
    return [
        {
            "xt": np.ascontiguousarray(x[b].T).astype(bf),
            "wqkvt": wqkvt,
            "woutt": woutt,
            "bias": bias,
        }
        for b in range(N_CORES)
    ]


def kernel(x, w_qkv, w_out, b_out):
    nc = _get_nc()
    in_maps = make_in_maps(x, w_qkv, w_out, b_out)
    res = run_bass_kernel_spmd(nc, in_maps, core_ids=list(range(N_CORES)))
    return np.stack(
        [res.results[b]["out"] for b in range(N_CORES)], axis=0
    ).astype(np.float32)
